# revision 19
# baseline (speedup 1.0000x reference)
"""Trainium2 Bass kernel for DeLanNet inverse dynamics.

out = tau_m + c1 + c2 + g   where per batch element (q, v=qDot, a2=qDDot):
  L = lower-tri from two MLPs on q, H = L L^T
  tau = L (L^T a2)
  c1  = 2 * (Dd p + Do u),  p = v*w, w = L^T v, u = outer-gathered v_i*w_j
  c2  = L alpha + A w,      A = dL/dq . v  (directional derivative)
  g   = MLP_g(q)

Key trick: the per-element Jacobians Dd[k,m] = d h_ld[m]/dq_k and
Do[k,n] = d h_lo[n]/dq_n are computed as a single matmul against
host-precomputed constant matrices:
  Dd = (1 - a_d^2) @ Gd  with Gd[h, m*7+k] = Wd1[k,h]*Wd2[h,m]
(1-sq)@Gd is folded as colsum(Gd) - sq@Gd; the colsum goes into a
constant bias row that is accumulated into PSUM by a rank-1 matmul
(ones[1,128] x bias_row[1,252]) at the start of each subtile's
accumulation group.

Sharding: pure data parallel over 8 cores (4096 batch elements each),
MLP weights/constants replicated.

Layout: "F-hidden": hidden activations live as [128 hid-chunk, Nb batch]
tiles so they are directly usable as matmul lhsT (stationary operand)
for the K=512 contractions, with no on-chip transposes anywhere.

Engine balance: tanh + PSUM->SBUF S-copies on ScalarE, squares +
reduces + small ops on DVE, several bilinear muls + the dense-A
copies on GpSimd (otherwise idle), matmuls on PE.
"""

import numpy as np

import concourse.bass as bass
import concourse.bacc as bacc
import concourse.mybir as mybir
import concourse.tile as tile
from concourse.bass_utils import run_bass_kernel_spmd

DOF = 7
HID = 512
B_FULL = 32768
N_CORES = 8
B_CORE = B_FULL // N_CORES  # 4096

F32 = mybir.dt.float32
BF16 = mybir.dt.bfloat16

# ---- tunables ----
import os
NB = int(os.environ.get("K_NB", "1024"))   # batch group size (multiple of 128)
DT_Z = os.environ.get("K_DT_Z", "bf16")    # first-layer matmul: bf16 | f32r | f32
DT_C = os.environ.get("K_DT_C", "bf16")    # contraction matmuls: bf16 | f32
A_BUFS = int(os.environ.get("K_A_BUFS", "26"))
SQ_BUFS = int(os.environ.get("K_SQ_BUFS", "18"))
Z_BUFS = int(os.environ.get("K_Z_BUFS", "2"))
S_BUFS = int(os.environ.get("K_S_BUFS", "4"))
SS_BUFS = int(os.environ.get("K_SS_BUFS", "2"))     # Ssb (group smalls) bufs
DT_T = os.environ.get("K_DT_T", "f32")              # mul-temp dtype: f32 | bf16
PIPE = int(os.environ.get("K_PIPE", "1"))
GP = int(os.environ.get("K_GP", "1"))       # gpsimd offload level (0..3)
# GP>=2 (u/t2/p/ad on GpSimd) and GP>=3 (tau/c2a/assembly) produce wrong
# results on HW — GpSimd mishandles some strided-view APs. Keep GP=1.
SCOPY = int(os.environ.get("K_SCOPY", "1")) # 0=DVE+bias 1=ScalarE+biasMM 2=alternate
N_SQ_GP = int(os.environ.get("K_N_SQ_GP", "2"))  # sq chunks on gpsimd
SQ_MODE = int(os.environ.get("K_SQ_MODE", "0"))  # 0=TT mul, 1=TS pow
HALF = int(os.environ.get("K_HALF", "2"))   # contraction/smalls splits per group
DT_MM = DT_C

_pairs_cm = [(i, j) for j in range(DOF - 1) for i in range(j + 1, DOF)]
_grp_base = [0]
for _j in range(6):
    _grp_base.append(_grp_base[-1] + (6 - _j))

# cblob column layout (bf16, [128, 1584])
_C_WLD = 0
_C_WLO = 196
_C_GDN = 392
_C_GON = 588
_C_WG2 = 1176
_C_BROW = 1204   # partition 0 only: bias row [252]
_C_ONES = 1456   # partition 0 only: ones [128]
_C_END = 1584


def _host_constants(Wd1, bd1, Wd2, bd2, Wo1, bo1, Wo2, bo2, Wg1, bg1, Wg2, bg2):
    TI, TJ = np.tril_indices(DOF, -1)
    orig_idx = np.array(
        [int(np.where((TI == i) & (TJ == j))[0][0]) for (i, j) in _pairs_cm]
    )
    Wo2_cm = Wo2[:, orig_idx]
    bo2_cm = bo2[orig_idx]

    W1cat = np.concatenate([Wd1, Wo1, Wg1], axis=1).astype(np.float32)  # [7,1536]
    b1cat = np.concatenate([bd1, bo1, bg1]).astype(np.float32)          # [1536]

    WL_d = np.zeros((HID, 49), np.float32)
    for m in range(DOF):
        WL_d[:, m * 7 + m] = Wd2[:, m]
    WL_o = np.zeros((HID, 49), np.float32)
    for n, (i, j) in enumerate(_pairs_cm):
        WL_o[:, i * 7 + j] = Wo2_cm[:, n]

    Gd_n = np.zeros((HID, 49), np.float32)   # negated Gd
    for m in range(DOF):
        for k in range(DOF):
            Gd_n[:, m * 7 + k] = -Wd1[k, :] * Wd2[:, m]
    Go_n = np.zeros((HID, 147), np.float32)  # negated Go
    for n in range(21):
        for k in range(DOF):
            Go_n[:, n * 7 + k] = -Wo1[k, :] * Wo2_cm[:, n]

    bias_row = np.zeros(252, np.float32)
    for m in range(DOF):
        bias_row[m * 7 + m] += bd2[m]
    for n, (i, j) in enumerate(_pairs_cm):
        bias_row[i * 7 + j] += bo2_cm[n]
    bias_row[49:98] = -Gd_n.sum(axis=0)
    bias_row[98:245] = -Go_n.sum(axis=0)
    bias_row[245:252] = bg2

    def chunkmaj(M):  # [512, N] -> [128, 4*N] with [p, c*N+n] = M[c*128+p, n]
        N = M.shape[1]
        return M.reshape(4, 128, N).transpose(1, 0, 2).reshape(128, 4 * N)

    import ml_dtypes
    np_c = np.float32 if DT_C == "f32" else ml_dtypes.bfloat16
    np_z = np.float32 if DT_Z != "bf16" else ml_dtypes.bfloat16

    cblob = np.zeros((128, _C_END), np.float32)
    cblob[:, _C_WLD:_C_WLD + 196] = chunkmaj(WL_d)
    cblob[:, _C_WLO:_C_WLO + 196] = chunkmaj(WL_o)
    cblob[:, _C_GDN:_C_GDN + 196] = chunkmaj(Gd_n)
    cblob[:, _C_GON:_C_GON + 588] = chunkmaj(Go_n)
    cblob[:, _C_WG2:_C_WG2 + 28] = chunkmaj(Wg2.astype(np.float32))
    cblob[0, _C_BROW:_C_BROW + 252] = bias_row
    cblob[0, _C_ONES:_C_ONES + 128] = 1.0

    fblob = np.zeros((128, 264), np.float32)
    fblob[:, 0:12] = b1cat.reshape(12, 128).T
    fblob[:, 12:264] = np.broadcast_to(bias_row, (128, 252))

    return {
        "W1cat": W1cat.astype(np_z),       # [7, 1536]
        "cblob": cblob.astype(np_c),       # [128, 1584]
        "fblob": fblob,                    # [128, 264] f32
    }, np_z


def build_bass():
    dt_c = F32 if DT_C == "f32" else BF16
    F32R = mybir.dt.float32r
    dt_z = {"f32": F32, "f32r": F32R, "bf16": BF16}[DT_Z]

    nc = bacc.Bacc("TRN2", target_bir_lowering=False, debug=False)

    x_s = nc.dram_tensor("x_s", [B_CORE, 21], F32, kind="ExternalInput").ap()
    xqT = nc.dram_tensor("xqT", [DOF, B_CORE], dt_z, kind="ExternalInput").ap()
    W1cat_d = nc.dram_tensor("W1cat", [DOF, 1536], dt_z, kind="ExternalInput").ap()
    cblob_d = nc.dram_tensor("cblob", [128, _C_END], dt_c, kind="ExternalInput").ap()
    fblob_d = nc.dram_tensor("fblob", [128, 264], F32, kind="ExternalInput").ap()
    out_s = nc.dram_tensor("out_s", [B_CORE, DOF], F32, kind="ExternalOutput").ap()

    dt_t = F32 if DT_T == "f32" else BF16
    NG = B_CORE // NB          # groups
    NS = NB // 128             # subtiles per group
    MUL = mybir.AluOpType.mult
    ADD = mybir.AluOpType.add

    with tile.TileContext(nc) as tc:
        import contextlib
        ctx = contextlib.ExitStack()
        with ctx:
            consts = ctx.enter_context(tc.tile_pool(name="consts", bufs=1))
            apool = ctx.enter_context(tc.tile_pool(name="apool", bufs=A_BUFS))
            sqpool = ctx.enter_context(tc.tile_pool(name="sqpool", bufs=SQ_BUFS))
            xq_pool = ctx.enter_context(tc.tile_pool(name="xqp", bufs=2))
            zpool = ctx.enter_context(tc.tile_pool(name="zp", bufs=Z_BUFS, space="PSUM"))
            spool = ctx.enter_context(tc.tile_pool(name="sp", bufs=S_BUFS, space="PSUM"))
            smalls = ctx.enter_context(tc.tile_pool(name="smalls", bufs=SS_BUFS))
            stmp = ctx.enter_context(tc.tile_pool(
                name="stmp", bufs=int(os.environ.get("K_ST_BUFS", "4"))))
            souts = ctx.enter_context(
                tc.tile_pool(name="souts", bufs=max(3, PIPE + 1)))

            # ---- constants into SBUF (3 DMAs; xq of group 0 first) ----
            W1_sb = consts.tile([DOF, 1536], dt_z)
            nc.sync.dma_start(out=W1_sb, in_=W1cat_d)
            xq0_sb = xq_pool.tile([DOF, NB], dt_z, tag="xq")
            nc.sync.dma_start(out=xq0_sb, in_=xqT[:, 0:NB])
            fblob_sb = consts.tile([128, 264], F32)
            nc.sync.dma_start(out=fblob_sb, in_=fblob_d)
            cblob_sb = consts.tile([128, _C_END], dt_c)
            nc.sync.dma_start(out=cblob_sb, in_=cblob_d)

            b1_sb = fblob_sb[:, 0:12]
            bias_sb = fblob_sb[:, 12:264]
            WLd_sb = cblob_sb[:, _C_WLD:_C_WLD + 196].rearrange(
                "p (c n) -> p c n", n=49)
            WLo_sb = cblob_sb[:, _C_WLO:_C_WLO + 196].rearrange(
                "p (c n) -> p c n", n=49)
            Gdn_sb = cblob_sb[:, _C_GDN:_C_GDN + 196].rearrange(
                "p (c n) -> p c n", n=49)
            Gon_sb = cblob_sb[:, _C_GON:_C_GON + 588].rearrange(
                "p (c n) -> p c n", n=147)
            Wg2_sb = cblob_sb[:, _C_WG2:_C_WG2 + 28].rearrange(
                "p (c n) -> p c n", n=7)
            brow_sb = cblob_sb[0:1, _C_BROW:_C_BROW + 252]
            ones_sb = cblob_sb[0:1, _C_ONES:_C_ONES + 128]

            NS2 = NS // HALF  # subtiles per contraction/smalls slice

            Az = consts.tile([128, NS2, 49], F32)  # dense A, zeros persist
            nc.vector.memset(Az, 0.0)

            def emit_z(gidx, xq_pre=None):
                b0 = gidx * NB

                # xqT slice for this group: [7, NB]
                if xq_pre is not None:
                    xq_sb = xq_pre
                else:
                    xq_sb = xq_pool.tile([DOF, NB], dt_z, tag="xq")
                    nc.sync.dma_start(out=xq_sb, in_=xqT[:, b0 : b0 + NB])

                # ---- phase 1: Z = W1^T.T @ xq -> tanh -> a (bf16), sq ----
                a_tiles = []
                sq_tiles = []
                for c in range(12):
                    zt = zpool.tile([128, NB], F32)  # PSUM
                    n512 = NB // 512
                    for jj in range(n512):
                        nc.tensor.matmul(
                            zt[:, jj * 512 : (jj + 1) * 512],
                            lhsT=W1_sb[:, c * 128 : (c + 1) * 128],
                            rhs=xq_sb[:, jj * 512 : (jj + 1) * 512],
                            start=True,
                            stop=True,
                        )
                    at = apool.tile([128, NB], dt_c, tag="a")
                    nc.scalar.activation(
                        at, zt, mybir.ActivationFunctionType.Tanh,
                        bias=b1_sb[:, c : c + 1], scale=1.0,
                    )
                    a_tiles.append(at)
                    if c < 8:
                        st = sqpool.tile([128, NB], dt_c, tag="sq")
                        if c < N_SQ_GP:
                            nc.gpsimd.tensor_mul(st, at, at)
                        elif SQ_MODE == 1:
                            nc.vector.tensor_scalar(
                                st, at, 2.0, None, mybir.AluOpType.pow)
                        else:
                            nc.vector.tensor_mul(st, at, at)
                        sq_tiles.append(st)

                # prefetch v|a2 slice for the smalls phase (one DMA)
                va2 = souts.tile([128, NS, 14], F32, tag="va2")
                xg = x_s[b0 : b0 + NB, :].rearrange("(t p) f -> p t f", p=128)
                nc.sync.dma_start(out=va2, in_=xg[:, :, 7:21])
                return a_tiles, sq_tiles, va2

            def emit_contraction(gidx, half, a_tiles, sq_tiles):
                # ---- phase 2: per-subtile contractions into PSUM [128, 252] ----
                Ssb = smalls.tile([128, NS2, 252], F32, tag="S")
                for si in range(NS2):
                    s = half * NS2 + si
                    bs = slice(s * 128, (s + 1) * 128)
                    ps = spool.tile([128, 252], F32)
                    use_scalar = (SCOPY == 1) or (SCOPY == 2 and s % 2 == 0)
                    if use_scalar:
                        # bias row via rank-1 matmul; must be the ONLY
                        # start=True in this accumulation group (start
                        # clears the whole bank's has_written state)
                        nc.tensor.matmul(
                            ps[:, 0:252], lhsT=ones_sb, rhs=brow_sb,
                            start=True, stop=False,
                        )
                    # L: cols 0:49  (a_d chunks then a_o chunks)
                    for c4 in range(4):
                        nc.tensor.matmul(
                            ps[:, 0:49], lhsT=a_tiles[c4][:, bs],
                            rhs=WLd_sb[:, c4, :],
                            start=(c4 == 0 and not use_scalar), stop=False,
                        )
                    for c4 in range(4):
                        nc.tensor.matmul(
                            ps[:, 0:49], lhsT=a_tiles[4 + c4][:, bs],
                            rhs=WLo_sb[:, c4, :], start=False, stop=(c4 == 3),
                        )
                    # Dd: cols 49:98  (sq_d)
                    for c4 in range(4):
                        nc.tensor.matmul(
                            ps[:, 49:98], lhsT=sq_tiles[c4][:, bs],
                            rhs=Gdn_sb[:, c4, :],
                            start=(c4 == 0 and not use_scalar), stop=(c4 == 3),
                        )
                    # Do: cols 98:245  (sq_o)
                    for c4 in range(4):
                        nc.tensor.matmul(
                            ps[:, 98:245], lhsT=sq_tiles[4 + c4][:, bs],
                            rhs=Gon_sb[:, c4, :],
                            start=(c4 == 0 and not use_scalar), stop=(c4 == 3),
                        )
                    # g: cols 245:252  (a_g)
                    for c4 in range(4):
                        nc.tensor.matmul(
                            ps[:, 245:252], lhsT=a_tiles[8 + c4][:, bs],
                            rhs=Wg2_sb[:, c4, :],
                            start=(c4 == 0 and not use_scalar), stop=(c4 == 3),
                        )
                    # PSUM -> SBUF
                    if use_scalar:
                        nc.scalar.activation(
                            Ssb[:, si, :], ps, mybir.ActivationFunctionType.Copy)
                    else:
                        nc.vector.tensor_add(Ssb[:, si, :], ps, bias_sb)
                return Ssb

            def emit_smalls(gidx, half, Ssb, va2):
                # ---- phase 3: smalls over [128, NS2, *] ----
                b0 = gidx * NB + half * NS2 * 128
                va2h = va2[:, half * NS2 : (half + 1) * NS2, :]
                vt = va2h[:, :, 0:7]
                a2t = va2h[:, :, 7:14]
                Lv = Ssb[:, :, 0:49].rearrange("p t (i j) -> p t i j", j=7)
                Dd = Ssb[:, :, 49:98].rearrange("p t (m k) -> p t m k", k=7)
                Do = Ssb[:, :, 98:245].rearrange("p t (n k) -> p t n k", k=7)
                g_t = Ssb[:, :, 245:252]

                v_mul = nc.gpsimd if GP >= 1 else nc.vector
                v_mul2 = nc.gpsimd if GP >= 3 else nc.vector
                v_cp = nc.gpsimd if GP >= 1 else nc.vector

                def bcast_inner(ap7):  # [128,NS,7] -> [128,NS,7(idx),7(bcast)]
                    return ap7.unsqueeze(3).broadcast_to((128, NS2, 7, 7))

                def bcast_outer(ap7):  # [128,NS,7] -> [128,NS,7(bcast),7(idx)]
                    return ap7.unsqueeze(2).broadcast_to((128, NS2, 7, 7))

                # fused w = L^T v and t1 = L^T a2: two muls write the
                # j-major transposed products into one [p,t,14,7] tile,
                # then ONE reduce over i yields [w | t1] as [p,t,14].
                LvT = Ssb[:, :, 0:49].rearrange("p t (i j) -> p t j i", j=7)
                t98 = stmp.tile([128, NS2, 14, 7], dt_t, tag="t98")
                wt1 = souts.tile([128, NS2, 14], F32, tag="wt1")
                nc.vector.tensor_mul(
                    t98[:, :, 0:7, :], LvT,
                    vt.unsqueeze(2).broadcast_to((128, NS2, 7, 7)),
                )
                nc.vector.tensor_mul(
                    t98[:, :, 7:14, :], LvT,
                    a2t.unsqueeze(2).broadcast_to((128, NS2, 7, 7)),
                )
                nc.vector.reduce_sum(wt1, t98, axis=mybir.AxisListType.X)
                w_t = wt1[:, :, 0:7]
                t1_t = wt1[:, :, 7:14]
                # tau = L t1
                t49c = stmp.tile([128, NS2, 7, 7], dt_t, tag="t49")
                tau_t = souts.tile([128, NS2, 7], F32, tag="tau")
                v_mul2.tensor_mul(t49c, Lv, bcast_outer(t1_t))
                nc.vector.reduce_sum(tau_t, t49c, axis=mybir.AxisListType.X)
                v_sm = nc.gpsimd if GP >= 2 else nc.vector
                # p = v*w
                p_t = souts.tile([128, NS2, 7], F32, tag="p")
                v_sm.tensor_mul(p_t, vt, w_t)
                # u: col-major gathered products
                u_t = souts.tile([128, NS2, 21], F32, tag="u")
                for j in range(6):
                    nb0 = _grp_base[j]
                    cnt = 6 - j
                    v_sm.tensor_mul(
                        u_t[:, :, nb0 : nb0 + cnt],
                        vt[:, :, j + 1 : 7],
                        w_t[:, :, j : j + 1].broadcast_to((128, NS2, cnt)),
                    )
                # c1d = Dd^T(p), c1o = Do^T(u)  (sum over m / n)
                t49d = stmp.tile([128, NS2, 7, 7], dt_t, tag="t49")
                c1d_t = souts.tile([128, NS2, 7], F32, tag="c1d")
                nc.vector.tensor_mul(t49d, Dd, bcast_inner(p_t))
                nc.vector.reduce_sum(
                    c1d_t, t49d.rearrange("p t m k -> p t k m"), axis=mybir.AxisListType.X
                )
                t147 = stmp.tile([128, NS2, 21, 7], dt_t, tag="t147")
                c1o_t = souts.tile([128, NS2, 7], F32, tag="c1o")
                v_mul.tensor_mul(
                    t147, Do, u_t[:].unsqueeze(3).broadcast_to((128, NS2, 21, 7))
                )
                nc.vector.reduce_sum(
                    c1o_t, t147.rearrange("p t n k -> p t k n"), axis=mybir.AxisListType.X
                )
                # dd = Dd v (sum over k), do = Do v
                t49e = stmp.tile([128, NS2, 7, 7], dt_t, tag="t49")
                dd_t = souts.tile([128, NS2, 7], F32, tag="dd")
                v_mul.tensor_mul(t49e, Dd, bcast_outer(vt))
                nc.vector.reduce_sum(dd_t, t49e, axis=mybir.AxisListType.X)
                t147b = stmp.tile([128, NS2, 21, 7], dt_t, tag="t147")
                do_t = souts.tile([128, NS2, 21], F32, tag="do")
                v_mul.tensor_mul(
                    t147b, Do, vt.unsqueeze(2).broadcast_to((128, NS2, 21, 7))
                )
                nc.vector.reduce_sum(do_t, t147b, axis=mybir.AxisListType.X)
                # alpha
                ad_t = souts.tile([128, NS2, 7], F32, tag="ad")
                v_sm.tensor_mul(ad_t, dd_t, vt)
                t2_t = souts.tile([128, NS2, 21], F32, tag="t2")
                al_t = souts.tile([128, NS2, 6], F32, tag="al")
                for j in range(6):
                    nb0 = _grp_base[j]
                    cnt = 6 - j
                    v_sm.tensor_mul(
                        t2_t[:, :, nb0 : nb0 + cnt],
                        do_t[:, :, nb0 : nb0 + cnt],
                        vt[:, :, j + 1 : 7],
                    )
                for j in range(6):
                    nb0 = _grp_base[j]
                    cnt = 6 - j
                    nc.vector.reduce_sum(
                        al_t[:, :, j : j + 1],
                        t2_t[:, :, nb0 : nb0 + cnt],
                        axis=mybir.AxisListType.X,
                    )
                alpha_t = souts.tile([128, NS2, 7], F32, tag="alpha")
                v_mul2.tensor_add(
                    alpha_t[:, :, 0:6], ad_t[:, :, 0:6], al_t[:, :, 0:6]
                )
                v_mul2.tensor_copy(alpha_t[:, :, 6:7], ad_t[:, :, 6:7])
                # c2a = L alpha
                t49f = stmp.tile([128, NS2, 7, 7], dt_t, tag="t49")
                c2a_t = souts.tile([128, NS2, 7], F32, tag="c2a")
                v_mul2.tensor_mul(t49f, Lv, bcast_outer(alpha_t))
                nc.vector.reduce_sum(c2a_t, t49f, axis=mybir.AxisListType.X)
                # build dense A (diag dd, lower do) in persistent zeroed Az
                diag_ap = bass.AP(
                    tensor=Az.tensor,
                    offset=Az.offset,
                    ap=[Az[:].ap[0], [49, NS2], [8, 7]],
                )
                v_cp.tensor_copy(diag_ap, dd_t)
                for j in range(6):
                    nb0 = _grp_base[j]
                    cnt = 6 - j
                    low_ap = bass.AP(
                        tensor=Az.tensor,
                        offset=Az.offset + (8 * j + 7),
                        ap=[Az[:].ap[0], [49, NS2], [7, cnt]],
                    )
                    v_cp.tensor_copy(low_ap, do_t[:, :, nb0 : nb0 + cnt])
                # c2b = A w
                t49g = stmp.tile([128, NS2, 7, 7], dt_t, tag="t49")
                c2b_t = souts.tile([128, NS2, 7], F32, tag="c2b")
                v_mul.tensor_mul(
                    t49g,
                    Az[:].rearrange("p t (i j) -> p t i j", j=7),
                    bcast_outer(w_t),
                )
                nc.vector.reduce_sum(c2b_t, t49g, axis=mybir.AxisListType.X)
                # assemble: out = tau + c2a + c2b + g + 2*(c1d+c1o)
                o1 = souts.tile([128, NS2, 7], F32, tag="o1")
                v_mul2.tensor_add(o1, tau_t, c2a_t)
                o2 = souts.tile([128, NS2, 7], F32, tag="o2")
                v_mul2.tensor_add(o2, o1, c2b_t)
                o3 = souts.tile([128, NS2, 7], F32, tag="o3")
                v_mul2.tensor_add(o3, o2, g_t)
                c1s = souts.tile([128, NS2, 7], F32, tag="c1s")
                nc.vector.tensor_add(c1s, c1d_t, c1o_t)
                of = souts.tile([128, NS2, 7], F32, tag="of")
                nc.vector.scalar_tensor_tensor(
                    of, in0=c1s, scalar=2.0, in1=o3, op0=MUL, op1=ADD
                )
                # store
                og = out_s[b0 : b0 + NS2 * 128, :].rearrange(
                    "(t p) f -> p t f", p=128)
                nc.sync.dma_start(out=og, in_=of)

            # software-pipelined emission, depth controlled by K_PIPE
            # emission: z one group ahead (PIPE>=1); contraction+smalls
            # run at half-group (NS2-subtile) granularity for a short tail
            zs = {}
            zs[0] = emit_z(0, xq0_sb)
            for g in range(NG):
                if PIPE >= 1 and g + 1 < NG:
                    zs[g + 1] = emit_z(g + 1)
                a_t, sq_t, va2_ = zs.pop(g)
                for h in range(HALF):
                    Ssb_ = emit_contraction(g, h, a_t, sq_t)
                    emit_smalls(g, h, Ssb_, va2_)
                if PIPE == 0 and g + 1 < NG:
                    zs[g + 1] = emit_z(g + 1)

    nc.compile()
    return nc


_CACHED = {}


def _make_in_maps(inputs):
    x = np.asarray(inputs["x"], np.float32)
    consts, np_mm = _host_constants(
        *[np.asarray(inputs[k], np.float32) for k in (
            "Wd1", "bd1", "Wd2", "bd2", "Wo1", "bo1", "Wo2", "bo2",
            "Wg1", "bg1", "Wg2", "bg2")]
    )
    xqT_full = np.ascontiguousarray(x[:, 0:DOF].T).astype(np_mm)
    in_maps = []
    for c in range(N_CORES):
        sl = slice(c * B_CORE, (c + 1) * B_CORE)
        m = {"x_s": np.ascontiguousarray(x[sl]),
             "xqT": np.ascontiguousarray(xqT_full[:, sl])}
        m.update(consts)
        in_maps.append(m)
    return in_maps


def kernel(**inputs):
    if "nc" not in _CACHED:
        _CACHED["nc"] = build_bass()
    nc = _CACHED["nc"]
    res = run_bass_kernel_spmd(
        nc, _make_in_maps(inputs), core_ids=list(range(N_CORES)))
    outs = [res.results[c]["out_s"] for c in range(N_CORES)]
    return np.concatenate(outs, axis=0).astype(np.float32)


def profile_once(inputs, tmpdir=None):
    """Run once with NTFF tracing; return device exec time in ns (or None)."""
    if "nc" not in _CACHED:
        _CACHED["nc"] = build_bass()
    nc = _CACHED["nc"]
    res = run_bass_kernel_spmd(
        nc, _make_in_maps(inputs), core_ids=list(range(N_CORES)),
        trace=True, tmpdir=tmpdir,
    )
    return res.exec_time_ns
